# revision 48
# baseline (speedup 1.0000x reference)
"""Causal self-attention with RoPE on 8 Trainium2 NeuronCores.

Sharding: batch x head-group. Core c handles batch b = c//2 and head group
g = c%2 (8 of 16 heads). Each core runs the full per-(batch, head-group)
pipeline on device; the host sums the two partial output projections per
batch (bf16) and adds b_out.

v3 (measured-cost-model rewrite; HW microbench: K=128 N=512 bf16 chained
matmuls stream at exactly N/2.4GHz=213ns with LDWEIGHTS fully hidden;
K=64 matmul pairs at lhsT bases 0/64 run concurrently via PE row tiling
at ~104ns each):
  - All DRAM inputs are host-pre-arranged partition-major so every DMA
    config is one fat descriptor per partition (8-32KB), loaded in a
    minimal-dependency order (wq, xT-chunk0, wk, wv first).
  - q and k live in one qkT tile [128, 2, PAIRS, L]; rotate_half's
    partition shuffle is 4 plain DMA partition-block copies per (c, mt)
    (sign folded into the host sin table), freeing the PE of the old
    signed-permutation matmuls and letting one DVE op rope q and k
    together.
  - Scores for the two heads of a pair go to one [128,2,512] PSUM tile as
    a K=64 row-tiled pair (concurrent on the array); one exp per (pr,kt)
    on ACT; multiplicative causal mask on DVE (diagonal tiles only);
    causally-shrunk column ranges everywhere.
  - PV: V carries a ones column (M=65) so PSUM row 64 accumulates the
    softmax denominator for free.
  - Epilogue per (c, pr): both PV accumulators (incl. den row) are copied
    PSUM->SBUF bf16 immediately, freeing the PV banks for the next pair;
    two M=64 selector matmuls broadcast the den rows to partitions 0-63
    of a 2-bank psum tile; ACT computes 1/x as exp(-ln(x)) on [64,2,512];
    two DVE muls write yT (upper head staged through a base-0 temp +
    DMA; elementwise engines cannot change partition base).
  - Fillers: chunk c's attention absorbs projection work for chunk c+1
    AND the output projection for chunk c-1 (paced interleave), so the
    ACT-heavy late chunks get PE filler and output DMA streams early.
  - Output is written bf16, one DMA per (lt, cc) tile; host accumulates
    in f32 and adds b_out.
"""

import os
import sys

if "/opt/trn_rl_repo" not in sys.path:
    sys.path.insert(0, "/opt/trn_rl_repo")

import numpy as np
import ml_dtypes

import concourse.bass as bass
import concourse.mybir as mybir
import concourse.tile as tile

F32 = mybir.dt.float32
BF16 = mybir.dt.bfloat16

B, L, D = 4, 2048, 1024
H, DH = 16, 64
NCORES = 8
G = 2                 # head groups (cores per batch)
HPC = H // G          # heads per core = 8
DQ = HPC * DH         # per-core q/k/v width = 512
PAIRS = HPC // 2      # 128-partition head pairs = 4
CHUNK = 512           # query-chunk (matmul free dim)
NCH = L // CHUNK      # 4
KT = D // 128         # 8 k-tiles over d_model
LT = L // 128         # 16 l-tiles
VW = DH + 1           # V columns per head incl. ones column = 65

LAST_RESULTS = None   # test harness reads perf fields from here


def legalize_bir_waits(bir_json: bytes) -> bytes:
    """Split multi-wait sync_infos into standalone EventSemaphore instrs.

    This container's walrus codegen accepts at most ONE sync wait per
    instruction (two for EventSemaphore), but Tile's sem assigner happily
    attaches several.  For every instruction carrying N>1 waits, keep one
    and hoist the rest onto EventSemaphore instructions inserted directly
    before it on the same engine (same block), which preserves each
    engine's program order and therefore the sync semantics.
    """
    import json as _json

    j = _json.loads(bir_json)
    uid = [0]
    for fn in j["functions"]:
        for blk in fn["blocks"]:
            out_insts = []
            for inst in blk["instructions"]:
                si = inst.get("sync_info")
                waits = (si or {}).get("on_wait") or []
                cap = 2 if inst.get("opcode") == "EventSemaphore" else 1
                if len(waits) > cap:
                    extra, keep = waits[:-cap], waits[-cap:]
                    for i in range(0, len(extra), 2):
                        uid[0] += 1
                        out_insts.append(
                            {
                                "name": f"antwaitfix-{uid[0]}",
                                "opcode": "EventSemaphore",
                                "engine": inst["engine"],
                                "ins": [],
                                "outs": [],
                                "debug": inst.get("debug", 0),
                                "sync_info": {
                                    "on_wait": extra[i : i + 2],
                                    "on_update": [],
                                },
                            }
                        )
                    si["on_wait"] = keep
                out_insts.append(inst)
            blk["instructions"] = out_insts
    return _json.dumps(j).encode()


def build_module():
    nc = bass.Bass(use_seq_codegen=True)

    xTp = nc.declare_dram_parameter("xTp", [128, KT * L], BF16, isOutput=False)
    wqp = nc.declare_dram_parameter("wqp", [128, KT * DQ], BF16, isOutput=False)
    wkp = nc.declare_dram_parameter("wkp", [128, KT * DQ], BF16, isOutput=False)
    wvp = nc.declare_dram_parameter("wvp", [128, KT * DQ], BF16, isOutput=False)
    wop = nc.declare_dram_parameter("wop", [128, PAIRS * D], BF16, isOutput=False)
    bq = nc.declare_dram_parameter("bq", [128, PAIRS], F32, isOutput=False)
    bk = nc.declare_dram_parameter("bk", [128, PAIRS], F32, isOutput=False)
    bv = nc.declare_dram_parameter("bv", [128, DQ], F32, isOutput=False)
    cosT = nc.declare_dram_parameter("cosT", [128, L], BF16, isOutput=False)
    sinT = nc.declare_dram_parameter("sinT", [128, L], BF16, isOutput=False)
    idm = nc.declare_dram_parameter("idm", [128, 128], BF16, isOutput=False)
    trim = nc.declare_dram_parameter("trim", [128, 128], BF16, isOutput=False)
    out = nc.declare_dram_parameter("out", [128, LT * 2 * CHUNK], BF16, isOutput=True)
    DBG = bool(os.environ.get("BASS_DEBUG_DUMP"))
    if DBG:
        qkd = nc.declare_dram_parameter(
            "qkd", [128, 2 * PAIRS * L], BF16, isOutput=True
        )
        vd = nc.declare_dram_parameter(
            "vd", [128, LT * HPC * VW], BF16, isOutput=True
        )
        ytd = nc.declare_dram_parameter(
            "ytd", [128, PAIRS * L], BF16, isOutput=True
        )

    xTr = xTp.rearrange("p (a l) -> p a l", a=KT)
    outr = out.rearrange("p (lt cc n) -> p lt cc n", cc=2, n=CHUNK)

    with tile.TileContext(nc) as tc:
        with (
            tc.tile_pool(name="const", bufs=1) as cp,
            tc.tile_pool(name="acts", bufs=1) as ap,
            tc.tile_pool(name="work", bufs=4) as wp,
            tc.tile_pool(name="sc", bufs=2, space="PSUM") as scp,
            tc.tile_pool(name="pv", bufs=2, space="PSUM") as pvp,
            tc.tile_pool(name="fp", bufs=2, space="PSUM") as fpp,
        ):
            # ---- input loads: one fat config per tensor (per-partition
            # contiguous DRAM), minimal-dependency order. Small constants
            # ride the ACT sequencer.
            xT_sb = ap.tile([128, KT, L], BF16)
            wq_sb = cp.tile([128, KT, DQ], BF16)
            wk_sb = cp.tile([128, KT, DQ], BF16)
            wv_sb = cp.tile([128, KT, DQ], BF16)
            wo_sb = cp.tile([128, PAIRS, D], BF16)
            bq_sb = cp.tile([128, PAIRS], F32)
            bk_sb = cp.tile([128, PAIRS], F32)
            bv_sb = cp.tile([128, DQ], F32)
            cos_sb = cp.tile([128, L], BF16)
            sin_sb = cp.tile([128, L], BF16)

            nc.sync.dma_start(xT_sb[:, :, 0:CHUNK], xTr[:, :, 0:CHUNK])
            nc.sync.dma_start(wq_sb.rearrange("p a b -> p (a b)")[:], wqp[:])
            nc.scalar.dma_start(cos_sb[:], cosT[:])
            nc.scalar.dma_start(sin_sb[:], sinT[:])
            nc.sync.dma_start(wk_sb.rearrange("p a b -> p (a b)")[:], wkp[:])
            nc.scalar.dma_start(bq_sb[:], bq[:])
            nc.sync.dma_start(wv_sb.rearrange("p a b -> p (a b)")[:], wvp[:])
            nc.scalar.dma_start(bk_sb[:], bk[:])
            nc.sync.dma_start(xT_sb[:, :, CHUNK : 2 * CHUNK],
                              xTr[:, :, CHUNK : 2 * CHUNK])
            nc.scalar.dma_start(bv_sb[:], bv[:])
            nc.sync.dma_start(xT_sb[:, :, 2 * CHUNK : L], xTr[:, :, 2 * CHUNK : L])
            nc.sync.dma_start(wo_sb.rearrange("p a b -> p (a b)")[:], wop[:])

            # Selector rows for the denominator-broadcast matmuls:
            # sel[:, 0, :] = [1]*64 + [0]*64, sel[:, 1, :] = its complement.
            sel_sb = cp.tile([128, 2, 128], BF16)
            nc.vector.memset(sel_sb[:, 0, 0:64], 1.0)
            nc.vector.memset(sel_sb[:, 0, 64:128], 0.0)
            nc.vector.memset(sel_sb[:, 1, 0:64], 0.0)
            nc.vector.memset(sel_sb[:, 1, 64:128], 1.0)
            id_sb = cp.tile([128, 128], BF16)
            tri01_sb = cp.tile([128, 128], BF16)
            nc.scalar.dma_start(id_sb[:], idm[:])
            nc.scalar.dma_start(tri01_sb[:], trim[:])

            # ~7us of zero-dependency garbage matmuls at t=0: the PE clock
            # gate (HAM) needs ~3.4us of sustained activity to release the
            # 1.2GHz throttle, and the real first matmuls are DMA-gated
            # until ~7us — so the prologue would otherwise run at half clock.
            wu_ps = fpp.tile([128, CHUNK], F32, tag="fp", name="warmup")
            for i in range(64):
                nc.tensor.matmul(
                    wu_ps[:, 0:256],
                    sel_sb[:, 0, :],
                    sel_sb.rearrange("p a b -> p (a b)")[:, 0:256],
                    start=True,
                    stop=True,
                )

            # q and k share one tile: qkT[:, 0] = q, qkT[:, 1] = k
            qkT_sb = ap.tile([128, 2, PAIRS, L], BF16)
            v_sb = ap.tile([128, LT, HPC * VW], BF16)
            yT_sb = ap.tile([128, PAIRS, L], BF16)
            # ones columns of V, set once for all l-tiles
            v4 = v_sb.rearrange("p lt (h c) -> p lt h c", c=VW)
            nc.vector.memset(v4[:, :, :, DH:VW], 1.0)

            def qk_group(c, mt, which):
                cs = slice(c * CHUNK, (c + 1) * CHUNK)
                qk = 0 if which == "q" else 1
                w_sb = wq_sb if which == "q" else wk_sb
                b_sb = bq_sb if which == "q" else bk_sb
                ps = fpp.tile(
                    [128, CHUNK], F32, tag="fp", name=f"{which}_{c}_{mt}"
                )
                for kt in range(KT):
                    nc.tensor.matmul(
                        ps[:],
                        w_sb[:, kt, mt * 128 : (mt + 1) * 128],
                        xT_sb[:, kt, cs],
                        start=(kt == 0),
                        stop=(kt == KT - 1),
                    )
                nc.vector.tensor_scalar_add(
                    qkT_sb[:, qk, mt, cs], ps[:], b_sb[:, mt : mt + 1]
                )

            def v_group(lt):
                ps = fpp.tile([128, CHUNK], F32, tag="fp", name=f"v_{lt}")
                for kt in range(KT):
                    nc.tensor.matmul(
                        ps[:],
                        xT_sb[:, kt, lt * 128 : (lt + 1) * 128],
                        wv_sb[:, kt, :],
                        start=(kt == 0),
                        stop=(kt == KT - 1),
                    )
                vdst = v_sb[:, lt, :].rearrange("p (h c) -> p h c", c=VW)
                nc.vector.tensor_add(vdst[:, :, 0:DH], ps[:], bv_sb[:])

            def rope_group(c, mt):
                # rotate_half = pure partition-block shuffle by DMA (the
                # sign is folded into the host sin table), then one DVE
                # mul/mul/add triple covering q and k together.
                cs = slice(c * CHUNK, (c + 1) * CHUNK)
                qk = qkT_sb[:, :, mt, cs]
                swp = wp.tile(
                    [128, 2, CHUNK], BF16, tag="swp", bufs=2,
                    name=f"swp_{c}_{mt}",
                )
                for i, (db, sb) in enumerate(
                    [(0, 32), (32, 0), (64, 96), (96, 64)]
                ):
                    eng = nc.sync if i % 2 == 0 else nc.scalar
                    eng.dma_start(
                        swp[db : db + 32, :, :],
                        qkT_sb[sb : sb + 32, :, mt, cs],
                    )
                sinb = sin_sb[:, cs].unsqueeze(1).broadcast_to([128, 2, CHUNK])
                cosb = cos_sb[:, cs].unsqueeze(1).broadcast_to([128, 2, CHUNK])
                nc.vector.tensor_mul(swp[:], swp[:], sinb)
                nc.vector.tensor_mul(qk, qk, cosb)
                nc.vector.tensor_add(qk, qk, swp[:])

            def proj_closures(c):
                fs = []
                for mt in range(PAIRS):
                    fs.append(lambda c=c, mt=mt: qk_group(c, mt, "q"))
                    fs.append(lambda c=c, mt=mt: qk_group(c, mt, "k"))
                    fs.append(lambda c=c, mt=mt: rope_group(c, mt))
                    if mt == 0:
                        for lt in range(4 * c, 4 * c + 4):
                            fs.append(lambda lt=lt: v_group(lt))
                return fs

            def outproj_group(lt, cc):
                ps = fpp.tile([128, CHUNK], F32, tag="fp", name=f"op_{lt}_{cc}")
                for pr in range(PAIRS):
                    nc.tensor.matmul(
                        ps[:],
                        yT_sb[:, pr, lt * 128 : (lt + 1) * 128],
                        wo_sb[:, pr, cc * CHUNK : (cc + 1) * CHUNK],
                        start=(pr == 0),
                        stop=(pr == PAIRS - 1),
                    )
                ob = wp.tile([128, CHUNK], BF16, tag="ob", bufs=2,
                             name=f"ob_{lt}_{cc}")
                nc.vector.tensor_copy(ob[:], ps[:])
                nc.sync.dma_start(outr[:, lt, cc, :], ob[:])

            def outproj_closures(c):
                return [
                    lambda lt=lt, cc=cc: outproj_group(lt, cc)
                    for lt in range(4 * c, 4 * c + 4)
                    for cc in range(2)
                ]

            def attn_scores(c, pr, kt):
                # columns col < k0-q0 are fully causal-masked (every key in
                # this tile sits above the query), so scores/exp/PV all
                # run on the shrunk column range [off, CHUNK). On diagonal
                # tiles the within-tile triangle gets a -240 additive bias
                # via one extra K=128 matmul accumulating into the scores
                # psum (exp then yields ~1e-13), keeping the causal mask
                # off the DVE and out of the exp->PV chain.
                q0 = c * CHUNK
                k0 = kt * 128
                off = max(0, k0 - q0)
                sct = scp.tile(
                    [128, 2, CHUNK], F32, tag="sc", name=f"sc_{c}_{pr}_{kt}"
                )
                diag = k0 >= q0
                for hh in range(2):
                    nc.tensor.matmul(
                        sct[:, hh, off:CHUNK],
                        qkT_sb[hh * 64 : (hh + 1) * 64, 1, pr, k0 : k0 + 128],
                        qkT_sb[
                            hh * 64 : (hh + 1) * 64, 0, pr, q0 + off : q0 + CHUNK
                        ],
                        start=True,
                        stop=True,
                    )
                ex = wp.tile(
                    [128, 2, CHUNK], BF16, tag="ex", bufs=8,
                    name=f"ex_{c}_{pr}_{kt}"
                )
                nc.scalar.activation(
                    ex[:, :, off:CHUNK],
                    sct[:, :, off:CHUNK],
                    mybir.ActivationFunctionType.Exp,
                    scale=float(1.0 / np.sqrt(DH)),
                )
                if diag:
                    mbc = (
                        tri01_sb[:, 0:128]
                        .unsqueeze(1)
                        .broadcast_to([128, 2, 128])
                    )
                    nc.vector.tensor_mul(
                        ex[:, :, off : off + 128], ex[:, :, off : off + 128],
                        mbc,
                    )
                return ex

            def attn_pv(c, pr, kt, ys, ex, n_lk):
                q0 = c * CHUNK
                off = max(0, kt * 128 - q0)
                for hh in range(2):
                    h = 2 * pr + hh
                    nc.tensor.matmul(
                        ys[hh][0:VW, off:CHUNK],
                        v_sb[:, kt, h * VW : (h + 1) * VW],
                        ex[:, hh, off:CHUNK],
                        start=(kt == 0),
                        stop=(kt == n_lk - 1),
                    )

            def attn_epilogue_a(c, pr, ys):
                # Copy both PV accumulators (rows 0..64, incl. the den row)
                # to SBUF bf16 right away so the PV banks free for the next
                # pair. The normalization (part B) is emitted a couple of
                # iterations later so the in-order PE never idles waiting
                # for these copies before its broadcast matmuls.
                ysb = [
                    wp.tile([VW, CHUNK], BF16, tag=f"ysb{hh}", bufs=2,
                            name=f"ysb{hh}_{c}_{pr}")
                    for hh in range(2)
                ]
                for hh in range(2):
                    nc.vector.tensor_copy(ysb[hh][:], ys[hh][0:VW, :])
                return ysb

            def attn_epilogue_b(c, pr, ysb):
                # Broadcast the two den rows to partitions 0-63 of a 2-bank
                # psum tile (M=64 selector matmuls), compute 1/x as
                # exp(-ln(x)) on ACT (shared table set; DVE has no divide),
                # and write yT with two DVE muls. The upper head goes
                # through a base-0 temp + DMA (elementwise engines cannot
                # change partition base).
                q0 = c * CHUNK
                bcps = scp.tile([128, 2, CHUNK], F32, tag="sc",
                                name=f"bc_{c}_{pr}")
                for hh in range(2):
                    nc.tensor.matmul(
                        bcps[0:64, hh, :],
                        sel_sb[64:65, 0, 0:64],
                        ysb[hh][64:65, :],
                        start=True,
                        stop=True,
                    )
                lnb = wp.tile([64, 2, CHUNK], F32, tag="lnb", bufs=2,
                              name=f"lnb_{c}_{pr}")
                nc.scalar.activation(
                    lnb[:], bcps[0:64, :, :], mybir.ActivationFunctionType.Ln
                )
                bcs = wp.tile([64, 2, CHUNK], F32, tag="bcs", bufs=2,
                              name=f"bcs_{c}_{pr}")
                nc.scalar.activation(
                    bcs[:], lnb[:], mybir.ActivationFunctionType.Exp,
                    scale=-1.0,
                )
                nc.vector.tensor_mul(
                    yT_sb[0:64, pr, q0 : q0 + CHUNK], ysb[0][0:64, :],
                    bcs[:, 0, :],
                )
                yt = wp.tile([64, CHUNK], BF16, tag="yt", bufs=2,
                             name=f"yt_{c}_{pr}")
                nc.vector.tensor_mul(yt[:], ysb[1][0:64, :], bcs[:, 1, :])
                nc.scalar.dma_start(yT_sb[64:128, pr, q0 : q0 + CHUNK], yt[:])

            def attn_chunk(c, fillers):
                """Emit chunk c's attention iterations with filler groups
                interleaved evenly (priority-spreading: the list scheduler
                prefers earlier-emitted work, so clustering fillers starves
                ACT of exp work while PE grinds through them)."""
                n_lk = 4 * (c + 1)
                n_iters = PAIRS * n_lk
                pace = len(fillers) / max(n_iters, 1)
                credit = 0.0
                pend = None  # deferred epilogue part B of the previous pair
                for pr in range(PAIRS):
                    ys = [
                        pvp.tile(
                            [128, CHUNK], F32, tag="pv", name=f"ys_{c}_{pr}_{hh}"
                        )
                        for hh in range(2)
                    ]
                    # software-pipelined, two kt per step: the four score
                    # matmuls of kt/kt+1 are emitted back-to-back (K=64
                    # row-tile concurrency only kicks in on runs >= 3
                    # alternating-base matmuls), then the four PV matmuls
                    # of the previous step, whose exps landed meanwhile.
                    pend_ex = []
                    for ktb in range(0, n_lk, 2):
                        e0 = attn_scores(c, pr, ktb)
                        e1 = attn_scores(c, pr, ktb + 1)
                        for kt_, e_ in pend_ex:
                            attn_pv(c, pr, kt_, ys, e_, n_lk)
                        pend_ex = [(ktb, e0), (ktb + 1, e1)]
                        if ktb == 2 and pend is not None:
                            attn_epilogue_b(c, pend[0], pend[1])
                            pend = None
                        credit += 2 * pace
                        if ktb == n_lk - 2:
                            # bias fillers toward the pair boundary, where
                            # the PE otherwise outruns the ACT exp backlog
                            credit += 1.5
                        while credit >= 1.0 and fillers:
                            fillers.popleft()()
                            credit -= 1.0
                    for kt_, e_ in pend_ex:
                        attn_pv(c, pr, kt_, ys, e_, n_lk)
                    ysb = attn_epilogue_a(c, pr, ys)
                    pend = (pr, ysb)
                while fillers:
                    fillers.popleft()()
                attn_epilogue_b(c, pend[0], pend[1])

            from collections import deque

            p0 = proj_closures(0)
            # startup: emit only what attention(0, pr=0, kt=0) needs (q0/k0/
            # rope0 + the first v tile); everything else is interleaved
            # filler. outproj(c) runs as filler inside chunk c+1 so the
            # ACT-heavy late chunks keep the PE fed and output DMA streams
            # early.
            for f in p0[:3]:
                f()
            attn_chunk(0, deque(p0[3:] + proj_closures(1)))
            attn_chunk(1, deque(proj_closures(2)))
            attn_chunk(2, deque(proj_closures(3) + outproj_closures(0)))
            attn_chunk(3, deque(outproj_closures(1) + outproj_closures(2)))
            # outproj(3) reads chunk-3 yT, which is only written by the
            # epilogues above — it cannot be filler inside chunk 3.
            for f in outproj_closures(3):
                f()
            if DBG:
                nc.sync.dma_start(
                    qkd[:], qkT_sb.rearrange("p a b c -> p (a b c)")[:]
                )
                nc.sync.dma_start(
                    vd[:], v_sb.rearrange("p a b -> p (a b)")[:]
                )
                nc.sync.dma_start(
                    ytd[:], yT_sb.rearrange("p a b -> p (a b)")[:]
                )
    return nc


def _rope_tables():
    inv_freq = (1.0 / (10000.0 ** (np.arange(0, DH, 2, dtype=np.float32) / DH))).astype(
        np.float32
    )
    t = np.arange(L, dtype=np.float32)
    freqs = np.einsum("l,d->ld", t, inv_freq).astype(np.float32)  # (L, 32)
    emb = np.concatenate([freqs, freqs], axis=-1)                 # (L, 64)
    cos = np.cos(emb).astype(np.float32)
    sin = np.sin(emb).astype(np.float32)
    cos128 = np.tile(cos.T, (2, 1))                # (128, L)
    sin128 = np.tile(sin.T, (2, 1))
    # rotate_half's sign rides the sin table: partitions with (p%64)<32
    # multiply the +32-shuffled value by -sin (rot[c] = -x[c+32] there).
    sgn = np.where((np.arange(128) % 64) < 32, -1.0, 1.0).astype(np.float32)
    sin128 = sin128 * sgn[:, None]
    return cos128, sin128


def _tri_neg():
    # trim[p, q] = 0 iff key p > query q (multiplicative causal mask for
    # the within-diagonal-tile triangle, applied on DVE after exp)
    p = np.arange(128)[:, None]
    q = np.arange(128)[None, :]
    return np.where(p > q, 0.0, 1.0).astype(np.float32)


def _bf16(a):
    return np.asarray(a, dtype=np.float32).astype(ml_dtypes.bfloat16)


def _pmajor(a, kt=KT):
    """[kt*128, n] -> [128, kt*n] partition-major (contiguous per partition)."""
    n = a.shape[1]
    return np.ascontiguousarray(
        a.reshape(kt, 128, n).transpose(1, 0, 2).reshape(128, kt * n)
    )


_COMPILED = None


def _ensure_trace_hook() -> bool:
    """Install the axon NTFF profile hook if the boot shim couldn't.

    The image's `antenv` stub lacks `axon_hooks`, so bass_utils' trace
    path crashes on import. Synthesize the module and wire in the ctypes
    hook from trn_agent_boot. Returns True iff tracing is usable.
    """
    try:
        from antenv.axon_hooks import get_axon_ntff_profile_hook  # noqa: F401

        return True
    except ImportError:
        pass
    try:
        import types

        import antenv
        import trn_agent_boot.trn_boot as tb

        mod = types.ModuleType("antenv.axon_hooks")
        _hook = [None]
        mod.set_axon_ntff_profile_hook = lambda h: _hook.__setitem__(0, h)
        mod.get_axon_ntff_profile_hook = lambda: _hook[0]
        sys.modules["antenv.axon_hooks"] = mod
        antenv.axon_hooks = mod
        mod.set_axon_ntff_profile_hook(
            tb._ntff_profile_via_ctypes("/opt/axon/libaxon_pjrt.so")
        )
        return True
    except Exception:
        return False


def kernel(x, pad_mask, W_qkv, b_qkv, W_out, b_out):
    global LAST_RESULTS, _COMPILED
    from concourse.bass_utils import run_bass_kernel_spmd

    x = np.asarray(x, dtype=np.float32)
    W_qkv = np.asarray(W_qkv, dtype=np.float32)
    b_qkv = np.asarray(b_qkv, dtype=np.float32)
    W_out = np.asarray(W_out, dtype=np.float32)
    b_out = np.asarray(b_out, dtype=np.float32)

    cos128, sin128 = _rope_tables()

    in_maps = []
    for core in range(NCORES):
        b, g = core // G, core % G
        sl = slice(g * DQ, (g + 1) * DQ)
        wq = W_qkv[:, 0 * D : 1 * D][:, sl]
        wk = W_qkv[:, 1 * D : 2 * D][:, sl]
        wv = W_qkv[:, 2 * D : 3 * D][:, sl]
        bqv = b_qkv[0 * D : 1 * D][sl]
        bkv = b_qkv[1 * D : 2 * D][sl]
        bvv = b_qkv[2 * D : 3 * D][sl]
        in_maps.append(
            {
                "xTp": _bf16(_pmajor(np.ascontiguousarray(x[b].T))),
                "wqp": _bf16(_pmajor(wq)),
                "wkp": _bf16(_pmajor(wk)),
                "wvp": _bf16(_pmajor(wv)),
                "wop": _bf16(_pmajor(W_out[sl, :], kt=PAIRS)),
                "bq": np.ascontiguousarray(bqv.reshape(PAIRS, 128).T),
                "bk": np.ascontiguousarray(bkv.reshape(PAIRS, 128).T),
                "bv": np.tile(bvv[None, :], (128, 1)).astype(np.float32),
                "cosT": _bf16(cos128),
                "sinT": _bf16(sin128),
                "idm": _bf16(np.eye(128, dtype=np.float32)),
                "trim": _bf16(_tri_neg()),
            }
        )

    if _COMPILED is None:
        nc = build_module()
        fixed = legalize_bir_waits(nc.to_json_bytes())
        nc.to_json_bytes = lambda: fixed  # bass2jax ships this BIR to walrus
        _COMPILED = nc
    nc = _COMPILED

    res = run_bass_kernel_spmd(
        nc,
        in_maps,
        core_ids=list(range(NCORES)),
        trace=bool(os.environ.get("BASS_TRACE")) and _ensure_trace_hook(),
    )
    LAST_RESULTS = res

    out = np.zeros((B, L, D), dtype=np.float32)
    for core in range(NCORES):
        o = np.asarray(res.results[core]["out"], dtype=np.float32)
        o = o.reshape(128, LT, 2, CHUNK).transpose(1, 0, 2, 3).reshape(L, D)
        out[core // G] += o
    out += b_out[None, None, :]
    return out
